# revision 61
# baseline (speedup 1.0000x reference)
"""BitLinear forward on 8 Trainium2 NeuronCores.

out = (x_q @ w_q) * (beta * gamma)
  a      = mean(weight);  w_q = sign(weight - a)
  gamma  = max|x| per row; x_q = clip(x/(gamma+eps), -(1-eps), 1-eps)
  beta   = max|weight|

Sharding: data-parallel over rows of x (N=32768 -> 4096 rows/core),
weight (1024x1024) replicated; per-core scalar stats are computed
redundantly so no collectives are needed.

Kernel math note: since QB == 1, (x_q @ w_q)*beta*gamma equals
(x @ w_q) * beta * gamma/(gamma+eps) up to the +-(1-eps) clip.  The clip
only affects the row-max element by <=1e-5 relative, and gamma/(gamma+eps)
deviates from 1 by <= eps/gamma ~ 4e-6 -- both far below the bf16 rounding
used for the matmul (~2e-3).  So the kernel never materializes x_q or even
gamma; it feeds bf16(x) to the tensor engine and multiplies the output by
the scalar beta.

Engine layout (steady state, one 128-row tile every ~4.5us):
  Pool  (SWDGE)    x-tile loads (queued behind the weight chunks, so the
                   weight -> mean -> sign critical path gets HBM first)
  DVE              fp32 -> bf16 cast of the x tile, fused with the beta
                   scaling (so PSUM holds the final fp32 output), then
                   evacuation of the transposed tile PSUM -> SBUF
  PE               8 transpose-mode matmuls (building xT in a PSUM bank)
                   + 16 matmuls (8 k-chunks x 2 PSUM half-banks).
                   The DMA xbar transpose was measured to serialize
                   against every other DMA copy in flight (~7us per
                   tile), so the transpose lives on the tensor engine.
  ACT              plain PSUM -> SBUF copy of the finished output tile
  SP    (HWDGE)    output stores
The 128x128 bf16 identity for the transposes is passed in as an extra
host-side input tensor.
"""

import sys

import numpy as np

if "/opt/trn_rl_repo" not in sys.path:
    sys.path.insert(0, "/opt/trn_rl_repo")

N_CORES = 8
N_FEAT = 1024
N_OUT = 1024
P = 128
KC = N_FEAT // P  # 8 contraction chunks of 128
EPS = 1e-5

_NC_CACHE = {}
_PATCHED = False


def _split_multi_waits(nc, max_waits=1):
    """The walrus build in this image rejects instructions carrying more
    than one sync-wait ("Too many sync wait commands").  Tile's semaphore
    assignment attaches one wait per producer proc, so hoist surplus waits
    onto NOP carrier instructions inserted immediately before the waiting
    instruction on the same engine (waits execute before the instruction
    body, so this preserves semantics exactly)."""
    import bass_rust

    for fn in nc.m.functions:
        for blk in fn.blocks:
            insts = blk.instructions  # live list
            i = 0
            while i < len(insts):
                ins = insts[i]
                si = getattr(ins, "sync_info", None)
                if si is None:
                    i += 1
                    continue
                waits = list(si.on_wait)
                if len(waits) <= max_waits:
                    i += 1
                    continue
                keep = waits[:max_waits]
                surplus = waits[max_waits:]
                si.on_wait = keep
                carriers = []
                cur_list = nc.cur_bb.bb.instructions
                for j in range(0, len(surplus), max_waits):
                    nop = nc.engines[ins.engine].nop(nofuse=True)
                    nop.ins.sync_info = bass_rust.SyncInfo(
                        on_wait=surplus[j : j + max_waits], on_update=[]
                    )
                    popped = cur_list.pop()
                    assert popped is nop.ins
                    carriers.append(nop.ins)
                for k, c in enumerate(carriers):
                    insts.insert(i + k, c)
                i += len(carriers) + 1


def _patch_tile_drain():
    global _PATCHED
    if _PATCHED:
        return
    _PATCHED = True
    import concourse.tile as tile

    orig = tile.TileContext._drain_and_barrier

    def patched(self, tick_clock, wait_clock):
        orig(self, tick_clock, wait_clock)
        _split_multi_waits(self.nc)

    tile.TileContext._drain_and_barrier = patched


def _build_nc(rows_per_core: int):
    import concourse.bass as bass
    import concourse.mybir as mybir
    import concourse.tile as tile

    _patch_tile_drain()

    f32 = mybir.dt.float32
    bf16 = mybir.dt.bfloat16
    R = rows_per_core
    assert R % P == 0
    T = R // P

    nc = bass.Bass("TRN2", target_bir_lowering=False, debug=False)
    x_h = nc.declare_dram_parameter("x", [R, N_FEAT], f32, isOutput=False)
    w_h = nc.declare_dram_parameter("weight", [N_FEAT, N_OUT], f32, isOutput=False)
    i_h = nc.declare_dram_parameter("ident", [P, P], bf16, isOutput=False)
    o_h = nc.declare_dram_parameter("out", [R, N_OUT], f32, isOutput=True)

    x_ap = x_h[:, :]
    o_ap = o_h[:, :]
    # weight[c*128 + p, n] -> [p, c, n]
    w_ap = w_h[:, :].rearrange("(c p) n -> p c n", p=P)

    with tile.TileContext(nc) as tc:
        with (
            tc.tile_pool(name="wpool", bufs=1) as wpool,
            tc.tile_pool(name="xpool", bufs=4) as xpool,
            tc.tile_pool(name="bpool", bufs=8) as bpool,
            tc.tile_pool(name="tpool", bufs=8) as tpool,
            tc.tile_pool(name="opool", bufs=6) as opool,
            tc.tile_pool(name="pspool", bufs=3, space="PSUM") as pspool,
            tc.tile_pool(name="ps1pool", bufs=2, space="PSUM") as ps1pool,
        ):
            # ---- weight preamble (all stats stay on-chip) ----
            w32 = wpool.tile([P, KC, N_OUT], f32, tag="w32")
            wq = wpool.tile([P, KC, N_OUT], bf16, tag="wq")
            wsum = wpool.tile([P, KC], f32, tag="wsum")
            wmax = wpool.tile([P, KC], f32, tag="wmax")
            ssum = wpool.tile([P, 1], f32, tag="ssum")
            bmax = wpool.tile([P, 1], f32, tag="bmax")
            pack2 = wpool.tile([1, 2], f32, tag="pack2")
            ones1 = wpool.tile([1, P], f32, tag="ones1")
            ones128 = wpool.tile([P, P], f32, tag="ones128")
            stats = wpool.tile([P, 2], f32, tag="stats")

            ident = wpool.tile([P, P], bf16, tag="ident")
            nc.sync.dma_start(out=ident, in_=i_h[:, :])
            nc.vector.memset(ones1, 1.0)
            nc.vector.memset(ones128, 1.0)

            def emit_x_chain(t):
                rows = slice(t * P, (t + 1) * P)
                x32 = xpool.tile([P, N_FEAT], f32, tag="x32")
                nc.gpsimd.dma_start(out=x32, in_=x_ap[rows, :])
                xb = bpool.tile([P, N_FEAT], bf16, tag="xb")
                nc.vector.tensor_copy(out=xb, in_=x32)
                # xT[p, c, r] = xb[r, c*128 + p] via 8 PE transposes into
                # one PSUM bank, then a single DVE evacuation
                xTps = ps1pool.tile([P, KC, P], bf16, tag="xTps")
                for c in range(KC):
                    nc.tensor.transpose(
                        xTps[:, c, :], xb[:, c * P : (c + 1) * P], ident
                    )
                xT = tpool.tile([P, KC, P], bf16, tag="xT")
                nc.vector.tensor_copy(out=xT, in_=xTps)
                return xT


            # weight chunks spread across all three DMA queues so their
            # per-DMA fixed overheads overlap and the 4MiB load runs at
            # HBM rate; x-tile prefetch sits behind them in the Pool FIFO
            w_engines = [nc.gpsimd, nc.scalar, nc.sync]
            for c in range(KC):
                w_engines[c % 3].dma_start(out=w32[:, c, :], in_=w_ap[:, c, :])
            for c in range(KC):
                # per-chunk row sums on ACT (accum_out); the copy itself is
                # a throwaway into wq, which sign() overwrites later
                nc.scalar.activation(
                    out=wq[:, c, :], in_=w32[:, c, :],
                    func=mybir.ActivationFunctionType.Copy,
                    bias=0.0, scale=1.0,
                    accum_out=wsum[:, c : c + 1],
                )
                nc.vector.tensor_reduce(
                    wmax[:, c : c + 1], w32[:, c, :],
                    axis=mybir.AxisListType.X, op=mybir.AluOpType.max,
                    apply_absolute_value=True,
                )
            # ---- mean fast path: one ones[128,128] matmul both reduces
            # across partitions AND replicates the total to all 128 output
            # partitions; no gpsimd C-reduce, no separate broadcast hop.
            # This chain (ACT sums -> ssum -> ones-MM -> scale) gates the
            # signs and therefore every matmul, so it is kept minimal.
            nc.vector.tensor_reduce(
                ssum, wsum, axis=mybir.AxisListType.X, op=mybir.AluOpType.add
            )
            na_ps = ps1pool.tile([P, 1], f32, tag="xTps")
            nc.tensor.matmul(na_ps, ones128, ssum, start=True, stop=True)
            nc.vector.tensor_scalar_mul(
                stats[:, 0:1], na_ps, -1.0 / float(N_FEAT * N_OUT)
            )
            neg_a = stats[:, 0:1]
            beta = stats[:, 1:2]

            # w_q = sign(w - a) immediately after the mean; the beta path
            # below runs in parallel (beta is only needed by the first
            # output evacuation, several microseconds later)
            for c in range(KC):
                nc.scalar.activation(
                    out=wq[:, c, :], in_=w32[:, c, :],
                    func=mybir.ActivationFunctionType.Sign,
                    bias=neg_a, scale=1.0,
                )

            # tile 0's x-chain: transposes run during the PE-idle window
            xT_next = emit_x_chain(0)

            # ---- beta slow path (max cannot ride a matmul) ----
            nc.vector.tensor_reduce(
                bmax, wmax, axis=mybir.AxisListType.X, op=mybir.AluOpType.max
            )
            nc.gpsimd.tensor_reduce(
                pack2[:, 1:2], bmax, axis=mybir.AxisListType.C,
                op=mybir.AluOpType.max,
            )
            b_ps = ps1pool.tile([P, 1], f32, tag="xTps")
            nc.tensor.matmul(b_ps, ones1, pack2[:, 1:2], start=True, stop=True)
            nc.vector.tensor_copy(out=stats[:, 1:2], in_=b_ps)

            # re-warm the PE clock right before the first real matmuls
            # (it idles during the weight load, so HAM throttles it)
            warm_ps = ps1pool.tile([P, P], bf16, tag="xTps")
            for _ in range(16):
                nc.tensor.transpose(warm_ps, ident, ident)

            # ---- main loop over 128-row tiles, transpose stage software-
            # pipelined one tile ahead: the PE stream becomes
            # [T8(t+1), MM16(t)], so the matmuls' wait on tile t's DVE
            # evacuation hides behind tile t+1's transposes ----
            for t in range(T):
                rows = slice(t * P, (t + 1) * P)

                xT = xT_next
                if t + 1 < T:
                    xT_next = emit_x_chain(t + 1)

                ps = pspool.tile([P, N_OUT], f32, tag="ps")
                for c in range(KC):
                    for h in range(2):
                        nc.tensor.matmul(
                            ps[:, h * 512 : (h + 1) * 512],
                            xT[:, c, :],
                            wq[:, c, h * 512 : (h + 1) * 512],
                            start=(c == 0),
                            stop=(c == KC - 1),
                        )

                o = opool.tile([P, N_OUT], f32, tag="o")
                nc.scalar.activation(
                    out=o, in_=ps,
                    func=mybir.ActivationFunctionType.Copy,
                    bias=0.0, scale=beta,
                )
                nc.sync.dma_start(out=o_ap[rows, :], in_=o)

    return nc


def _get_nc(rows_per_core: int):
    if rows_per_core not in _NC_CACHE:
        _NC_CACHE[rows_per_core] = _build_nc(rows_per_core)
    return _NC_CACHE[rows_per_core]


def run(x, weight, trace=False, trace_cores=None):
    """Run on 8 cores; returns (out, BassKernelResults)."""
    from concourse.bass_utils import run_bass_kernel_spmd

    import ml_dtypes

    x = np.ascontiguousarray(np.asarray(x, dtype=np.float32))
    weight = np.ascontiguousarray(np.asarray(weight, dtype=np.float32))
    ident = np.eye(P, dtype=ml_dtypes.bfloat16)
    n = x.shape[0]
    assert n % N_CORES == 0
    rpc = n // N_CORES
    nc = _get_nc(rpc)
    in_maps = [
        {"x": x[i * rpc : (i + 1) * rpc], "weight": weight, "ident": ident}
        for i in range(N_CORES)
    ]
    kwargs = {}
    if trace:
        kwargs["trace"] = True
        if trace_cores is not None:
            kwargs["trace_cores"] = trace_cores
    res = run_bass_kernel_spmd(nc, in_maps, core_ids=list(range(N_CORES)), **kwargs)
    out = np.concatenate([r["out"] for r in res.results], axis=0)
    return out, res


def kernel(x, weight):
    out, _ = run(x, weight)
    return out


# revision 62
# speedup vs baseline: 1.0156x; 1.0156x over previous
"""BitLinear forward on 8 Trainium2 NeuronCores.

out = (x_q @ w_q) * (beta * gamma)
  a      = mean(weight);  w_q = sign(weight - a)
  gamma  = max|x| per row; x_q = clip(x/(gamma+eps), -(1-eps), 1-eps)
  beta   = max|weight|

Sharding: data-parallel over rows of x (N=32768 -> 4096 rows/core),
weight (1024x1024) replicated; per-core scalar stats are computed
redundantly so no collectives are needed.

Kernel math note: since QB == 1, (x_q @ w_q)*beta*gamma equals
(x @ w_q) * beta * gamma/(gamma+eps) up to the +-(1-eps) clip.  The clip
only affects the row-max element by <=1e-5 relative, and gamma/(gamma+eps)
deviates from 1 by <= eps/gamma ~ 4e-6 -- both far below the bf16 rounding
used for the matmul (~2e-3).  So the kernel never materializes x_q or even
gamma; it feeds bf16(x) to the tensor engine and multiplies the output by
the scalar beta.

Engine layout (steady state, one 128-row tile every ~4.5us):
  Pool  (SWDGE)    x-tile loads (queued behind the weight chunks, so the
                   weight -> mean -> sign critical path gets HBM first)
  DVE              fp32 -> bf16 cast of the x tile, fused with the beta
                   scaling (so PSUM holds the final fp32 output), then
                   evacuation of the transposed tile PSUM -> SBUF
  PE               8 transpose-mode matmuls (building xT in a PSUM bank)
                   + 16 matmuls (8 k-chunks x 2 PSUM half-banks).
                   The DMA xbar transpose was measured to serialize
                   against every other DMA copy in flight (~7us per
                   tile), so the transpose lives on the tensor engine.
  ACT              plain PSUM -> SBUF copy of the finished output tile
  SP    (HWDGE)    output stores
The 128x128 bf16 identity for the transposes is passed in as an extra
host-side input tensor.
"""

import sys

import numpy as np

if "/opt/trn_rl_repo" not in sys.path:
    sys.path.insert(0, "/opt/trn_rl_repo")

N_CORES = 8
N_FEAT = 1024
N_OUT = 1024
P = 128
KC = N_FEAT // P  # 8 contraction chunks of 128
EPS = 1e-5

_NC_CACHE = {}
_PATCHED = False


def _split_multi_waits(nc, max_waits=1):
    """The walrus build in this image rejects instructions carrying more
    than one sync-wait ("Too many sync wait commands").  Tile's semaphore
    assignment attaches one wait per producer proc, so hoist surplus waits
    onto NOP carrier instructions inserted immediately before the waiting
    instruction on the same engine (waits execute before the instruction
    body, so this preserves semantics exactly)."""
    import bass_rust

    for fn in nc.m.functions:
        for blk in fn.blocks:
            insts = blk.instructions  # live list
            i = 0
            while i < len(insts):
                ins = insts[i]
                si = getattr(ins, "sync_info", None)
                if si is None:
                    i += 1
                    continue
                waits = list(si.on_wait)
                if len(waits) <= max_waits:
                    i += 1
                    continue
                keep = waits[:max_waits]
                surplus = waits[max_waits:]
                si.on_wait = keep
                carriers = []
                cur_list = nc.cur_bb.bb.instructions
                for j in range(0, len(surplus), max_waits):
                    nop = nc.engines[ins.engine].nop(nofuse=True)
                    nop.ins.sync_info = bass_rust.SyncInfo(
                        on_wait=surplus[j : j + max_waits], on_update=[]
                    )
                    popped = cur_list.pop()
                    assert popped is nop.ins
                    carriers.append(nop.ins)
                for k, c in enumerate(carriers):
                    insts.insert(i + k, c)
                i += len(carriers) + 1


def _patch_tile_drain():
    global _PATCHED
    if _PATCHED:
        return
    _PATCHED = True
    import concourse.tile as tile

    orig = tile.TileContext._drain_and_barrier

    def patched(self, tick_clock, wait_clock):
        orig(self, tick_clock, wait_clock)
        _split_multi_waits(self.nc)

    tile.TileContext._drain_and_barrier = patched


def _build_nc(rows_per_core: int):
    import concourse.bass as bass
    import concourse.mybir as mybir
    import concourse.tile as tile

    _patch_tile_drain()

    f32 = mybir.dt.float32
    bf16 = mybir.dt.bfloat16
    R = rows_per_core
    assert R % P == 0
    T = R // P

    nc = bass.Bass("TRN2", target_bir_lowering=False, debug=False)
    x_h = nc.declare_dram_parameter("x", [R, N_FEAT], f32, isOutput=False)
    w_h = nc.declare_dram_parameter("weight", [N_FEAT, N_OUT], f32, isOutput=False)
    i_h = nc.declare_dram_parameter("ident", [P, P], bf16, isOutput=False)
    o_h = nc.declare_dram_parameter("out", [R, N_OUT], f32, isOutput=True)

    x_ap = x_h[:, :]
    o_ap = o_h[:, :]
    # weight[c*128 + p, n] -> [p, c, n]
    w_ap = w_h[:, :].rearrange("(c p) n -> p c n", p=P)

    with tile.TileContext(nc) as tc:
        with (
            tc.tile_pool(name="wpool", bufs=1) as wpool,
            tc.tile_pool(name="xpool", bufs=3) as xpool,
            tc.tile_pool(name="bpool", bufs=8) as bpool,
            tc.tile_pool(name="tpool", bufs=8) as tpool,
            tc.tile_pool(name="opool", bufs=6) as opool,
            tc.tile_pool(name="pspool", bufs=3, space="PSUM") as pspool,
            tc.tile_pool(name="ps1pool", bufs=2, space="PSUM") as ps1pool,
        ):
            # ---- weight preamble (all stats stay on-chip) ----
            w32 = wpool.tile([P, KC, N_OUT], f32, tag="w32")
            wq = wpool.tile([P, KC, N_OUT], bf16, tag="wq")
            wsum = wpool.tile([P, KC], f32, tag="wsum")
            wmax = wpool.tile([P, KC], f32, tag="wmax")
            ssum = wpool.tile([P, 1], f32, tag="ssum")
            bmax = wpool.tile([P, 1], f32, tag="bmax")
            pack2 = wpool.tile([1, 2], f32, tag="pack2")
            ones1 = wpool.tile([1, P], f32, tag="ones1")
            ones128 = wpool.tile([P, P], f32, tag="ones128")
            stats = wpool.tile([P, 2], f32, tag="stats")

            ident = wpool.tile([P, P], bf16, tag="ident")
            nc.sync.dma_start(out=ident, in_=i_h[:, :])
            nc.vector.memset(ones1, 1.0)
            nc.vector.memset(ones128, 1.0)

            def emit_x_chain(t):
                rows = slice(t * P, (t + 1) * P)
                x32 = xpool.tile([P, N_FEAT], f32, tag="x32")
                nc.gpsimd.dma_start(out=x32, in_=x_ap[rows, :])
                xb = bpool.tile([P, N_FEAT], bf16, tag="xb")
                nc.vector.tensor_copy(out=xb, in_=x32)
                # xT[p, c, r] = xb[r, c*128 + p] via 8 PE transposes into
                # one PSUM bank, then a single DVE evacuation
                xTps = ps1pool.tile([P, KC, P], bf16, tag="xTps")
                for c in range(KC):
                    nc.tensor.transpose(
                        xTps[:, c, :], xb[:, c * P : (c + 1) * P], ident
                    )
                xT = tpool.tile([P, KC, P], bf16, tag="xT")
                nc.vector.tensor_copy(out=xT, in_=xTps)
                return xT


            # weight chunks spread across all three DMA queues so their
            # per-DMA fixed overheads overlap and the 4MiB load runs at
            # HBM rate; x-tile prefetch sits behind them in the Pool FIFO
            w_engines = [nc.gpsimd, nc.scalar, nc.sync]
            for c in range(KC):
                w_engines[c % 3].dma_start(out=w32[:, c, :], in_=w_ap[:, c, :])
            for c in range(KC):
                # per-chunk row sums on ACT (accum_out); the copy itself is
                # a throwaway into wq, which sign() overwrites later
                nc.scalar.activation(
                    out=wq[:, c, :], in_=w32[:, c, :],
                    func=mybir.ActivationFunctionType.Copy,
                    bias=0.0, scale=1.0,
                    accum_out=wsum[:, c : c + 1],
                )
                nc.vector.tensor_reduce(
                    wmax[:, c : c + 1], w32[:, c, :],
                    axis=mybir.AxisListType.X, op=mybir.AluOpType.max,
                    apply_absolute_value=True,
                )
            # ---- mean fast path: one ones[128,128] matmul both reduces
            # across partitions AND replicates the total to all 128 output
            # partitions; no gpsimd C-reduce, no separate broadcast hop.
            # This chain (ACT sums -> ssum -> ones-MM -> scale) gates the
            # signs and therefore every matmul, so it is kept minimal.
            nc.vector.tensor_reduce(
                ssum, wsum, axis=mybir.AxisListType.X, op=mybir.AluOpType.add
            )
            na_ps = ps1pool.tile([P, 1], f32, tag="xTps")
            nc.tensor.matmul(na_ps, ones128, ssum, start=True, stop=True)
            nc.vector.tensor_scalar_mul(
                stats[:, 0:1], na_ps, -1.0 / float(N_FEAT * N_OUT)
            )
            neg_a = stats[:, 0:1]
            beta = stats[:, 1:2]

            # w_q = sign(w - a) immediately after the mean; the beta path
            # below runs in parallel (beta is only needed by the first
            # output evacuation, several microseconds later)
            for c in range(KC):
                nc.scalar.activation(
                    out=wq[:, c, :], in_=w32[:, c, :],
                    func=mybir.ActivationFunctionType.Sign,
                    bias=neg_a, scale=1.0,
                )

            # tile 0's x-chain: transposes run during the PE-idle window
            xT_next = emit_x_chain(0)

            # ---- beta slow path (max cannot ride a matmul) ----
            nc.vector.tensor_reduce(
                bmax, wmax, axis=mybir.AxisListType.X, op=mybir.AluOpType.max
            )
            nc.gpsimd.tensor_reduce(
                pack2[:, 1:2], bmax, axis=mybir.AxisListType.C,
                op=mybir.AluOpType.max,
            )
            b_ps = ps1pool.tile([P, 1], f32, tag="xTps")
            nc.tensor.matmul(b_ps, ones1, pack2[:, 1:2], start=True, stop=True)
            nc.vector.tensor_copy(out=stats[:, 1:2], in_=b_ps)

            # re-warm the PE clock right before the first real matmuls
            # (it idles during the weight load, so HAM throttles it)
            warm_ps = ps1pool.tile([P, P], bf16, tag="xTps")
            for _ in range(16):
                nc.tensor.transpose(warm_ps, ident, ident)

            # ---- main loop over 128-row tiles, transpose stage software-
            # pipelined one tile ahead: the PE stream becomes
            # [T8(t+1), MM16(t)], so the matmuls' wait on tile t's DVE
            # evacuation hides behind tile t+1's transposes ----
            for t in range(T):
                rows = slice(t * P, (t + 1) * P)

                xT = xT_next
                if t + 1 < T:
                    xT_next = emit_x_chain(t + 1)

                ps = pspool.tile([P, N_OUT], f32, tag="ps")
                for c in range(KC):
                    for h in range(2):
                        nc.tensor.matmul(
                            ps[:, h * 512 : (h + 1) * 512],
                            xT[:, c, :],
                            wq[:, c, h * 512 : (h + 1) * 512],
                            start=(c == 0),
                            stop=(c == KC - 1),
                        )

                o = opool.tile([P, N_OUT], f32, tag="o")
                nc.scalar.activation(
                    out=o, in_=ps,
                    func=mybir.ActivationFunctionType.Copy,
                    bias=0.0, scale=beta,
                )
                nc.sync.dma_start(out=o_ap[rows, :], in_=o)

    return nc


def _get_nc(rows_per_core: int):
    if rows_per_core not in _NC_CACHE:
        _NC_CACHE[rows_per_core] = _build_nc(rows_per_core)
    return _NC_CACHE[rows_per_core]


def run(x, weight, trace=False, trace_cores=None):
    """Run on 8 cores; returns (out, BassKernelResults)."""
    from concourse.bass_utils import run_bass_kernel_spmd

    import ml_dtypes

    x = np.ascontiguousarray(np.asarray(x, dtype=np.float32))
    weight = np.ascontiguousarray(np.asarray(weight, dtype=np.float32))
    ident = np.eye(P, dtype=ml_dtypes.bfloat16)
    n = x.shape[0]
    assert n % N_CORES == 0
    rpc = n // N_CORES
    nc = _get_nc(rpc)
    in_maps = [
        {"x": x[i * rpc : (i + 1) * rpc], "weight": weight, "ident": ident}
        for i in range(N_CORES)
    ]
    kwargs = {}
    if trace:
        kwargs["trace"] = True
        if trace_cores is not None:
            kwargs["trace_cores"] = trace_cores
    res = run_bass_kernel_spmd(nc, in_maps, core_ids=list(range(N_CORES)), **kwargs)
    out = np.concatenate([r["out"] for r in res.results], axis=0)
    return out, res


def kernel(x, weight):
    out, _ = run(x, weight)
    return out
